# revision 20
# baseline (speedup 1.0000x reference)
"""Trainium2 Bass kernel for nn_MoELayerStacks (moe_routing).

Full inputs in, full output out. Data-parallel over batch across 8 cores.

Math (per batch row b):
  gate = [x[:32], x[1536:1568]] @ router_w.T + router_b           # [8]
  idx  = argmax(gate)
  l1c  = x @ l1_w[e].T + l1_b[e]   for all e                      # [8, 16]
  l1x  = clip([square(l1c[:, :15])*255/256, l1c[:, :15]], 0, 1)   # [8, 30]
  l2x  = clip(l1x @ l2_w[e].T + l2_b[e], 0, 1)                    # [8, 32]
  out  = (l2x @ out_w[e].T + out_b[e] + l1c[:, 15])[idx]          # [1]

The problem is HBM-bound on reading x (201 MB fp32), so x and all matmul
weights are downcast to fp16 on the host: halves DMA traffic and runs the
PE at full rate (1 cyc/row). Verified rel-err ~3e-4 (budget 2e-2).

The router must match fp32 argmax bit-for-bit, so the 64 router features
are shipped as an fp16 hi/lo split (hi = fp16(x), lo = fp16(x - hi)) and
the gate is computed as whi@hi + wlo@hi + whi@lo in fp32 PSUM — max gate
error ~2e-6, zero argmax flips, same DMA bytes as one fp32 copy.

Layout: features on partitions ("transposed"), batch on the free dim.
Stacked l1 feature index: f = e for o=15 (the l1x_out features), and
f = 8 + o*8 + e for o in 0..14. l1's bias is folded into the PSUM
accumulation via a ones-row matmul. l2 outputs are split into two expert
groups (0-3, 4-7) of 128 features. The final argmax-gather runs in
batch-on-partitions layout after small PE transposes of the [8, mb]
gate / all-expert-output tiles.

Block plan per core: [512, 512, 512, 256, 256] batch rows. Each block's
l1 chunk matmuls are split around the previous block's tail so the PE has
fill work while DMA streams; the two trailing 256-blocks shrink the
serial compute tail after the last x byte lands.
"""

import os
from contextlib import ExitStack

import numpy as np

import concourse.bacc as bacc
import concourse.mybir as mybir
import concourse.tile as tile

N_CORES = 8
B, L1, L2, L3, E = 16384, 3072, 15, 32, 8
RF = 32  # router feats per perspective
HALF = L1 // 2
B_SH = B // N_CORES  # 2048 rows per core
KC = L1 // 128  # 24 contraction chunks
SQ_SCALE = float(np.sqrt(255.0 / 256.0))  # pre-square scale on l1 features
B_PLAN = [512, 512, 512, 256, 256]  # per-core batch blocks

F32 = mybir.dt.float32
F16 = mybir.dt.float16
ALU = mybir.AluOpType


def build_nc():
    nc = bacc.Bacc(dynamic_dma_scratch_size=2048)

    xT = nc.dram_tensor("xT", [L1, B_SH], F16, kind="ExternalInput")
    xr = nc.dram_tensor("xr", [128, B_SH], F16, kind="ExternalInput")
    w1t = nc.dram_tensor("w1t", [128, KC * 128], F16, kind="ExternalInput")
    wrp = nc.dram_tensor("wrp", [64, E], F16, kind="ExternalInput")
    wr2 = nc.dram_tensor("wr2", [64, E], F16, kind="ExternalInput")
    w2p = nc.dram_tensor("w2p", [128, 512], F16, kind="ExternalInput")
    w3p = nc.dram_tensor("w3p", [128, 16], F16, kind="ExternalInput")
    biasp = nc.dram_tensor("biasp", [128, 6], F32, kind="ExternalInput")
    idn = nc.dram_tensor("idn", [40, 40], F32, kind="ExternalInput")
    y = nc.dram_tensor("y", [128, B_SH // 128], F32, kind="ExternalOutput")

    moffs = np.cumsum([0] + B_PLAN).tolist()  # batch-row offset per block
    noffs = [m // 128 for m in moffs]  # y column offset per block
    nb = len(B_PLAN)

    with tile.TileContext(nc) as tc, ExitStack() as ctx:
        const = ctx.enter_context(tc.tile_pool(name="const", bufs=1))
        xp512 = ctx.enter_context(tc.tile_pool(name="xp512", bufs=12))
        xp256 = ctx.enter_context(tc.tile_pool(name="xp256", bufs=12))
        xrp = ctx.enter_context(tc.tile_pool(name="xrp", bufs=5))
        actp = ctx.enter_context(tc.tile_pool(name="act", bufs=2))
        smallp = ctx.enter_context(tc.tile_pool(name="small", bufs=2))
        ps_big = ctx.enter_context(tc.tile_pool(name="ps1", bufs=2, space="PSUM"))
        ps_gate = ctx.enter_context(tc.tile_pool(name="psg", bufs=2, space="PSUM"))
        ps_2a = ctx.enter_context(tc.tile_pool(name="ps2a", bufs=1, space="PSUM"))
        ps_2b = ctx.enter_context(tc.tile_pool(name="ps2b", bufs=1, space="PSUM"))
        ps_3 = ctx.enter_context(tc.tile_pool(name="ps3", bufs=1, space="PSUM"))
        ps_t = ctx.enter_context(tc.tile_pool(name="pst", bufs=1, space="PSUM"))

        # --- constants ---
        w1t_sb = const.tile([128, KC, 128], F16)
        w1t_v = w1t[:, :].rearrange("p (c f) -> p c f", f=128)

        def load_w1t(eng, c0, n):
            eng.dma_start(w1t_sb[:, c0 : c0 + n, :], w1t_v[:, c0 : c0 + n, :])

        w2_sb = const.tile([128, 512], F16)
        w3_sb = const.tile([128, 16], F16)
        wr_sb = const.tile([64, E], F16)
        wr2_sb = const.tile([64, E], F16)
        bias_sb = const.tile([128, 6], F32)
        idn_sb = const.tile([40, 40], F32)

        def emit_small_consts(eng):
            eng.dma_start(wr_sb[:], wrp[:, :])
            eng.dma_start(wr2_sb[:], wr2[:, :])
            eng.dma_start(w2_sb[:], w2p[:, :])
            eng.dma_start(w3_sb[:], w3p[:, :])
            eng.dma_start(bias_sb[:], biasp[:, :])
            eng.dma_start(idn_sb[:], idn[:, :])

        st = {}  # per-block live tiles for the skewed pipeline

        def load_xr(b, eng):
            # hi and lo in separate partition-0 tiles: the PE cannot take
            # matmul inputs at a nonzero partition offset (HW limitation)
            mb, m0 = B_PLAN[b], moffs[b]
            xh = xrp.tile([64, 512], F16, tag="xh", name="xh")[:, 0:mb]
            eng.dma_start(xh, xr[0:64, m0 : m0 + mb])
            xl = xrp.tile([64, 512], F16, tag="xl", name="xl")[:, 0:mb]
            eng.dma_start(xl, xr[64:128, m0 : m0 + mb])
            st[b]["xh"], st[b]["xl"] = xh, xl

        def load_piece(pool, tag, m0, mb, c0, n, nmax, eng):
            xt = pool.tile([128, nmax, mb], F16, tag=tag, name=tag)
            eng.dma_start(
                xt[:, 0:n, :],
                xT[c0 * 128 : (c0 + n) * 128, m0 : m0 + mb].rearrange(
                    "(c p) m -> p c m", p=128
                ),
            )
            return xt

        engs = [nc.sync, nc.scalar]

        def emit_load(b, first=False, last=False):
            mb, m0 = B_PLAN[b], moffs[b]
            st[b] = {"cmap": [None] * KC}

            if mb == 512:
                pool, tag, pn = xp512, "xt", 6
            else:
                pool, tag, pn = xp256, "xt2", (3 if last else 6)

            def add(c0, n, eng):
                xt = load_piece(pool, tag, m0, mb, c0, n, 6, eng)
                for k in range(n):
                    st[b]["cmap"][c0 + k] = xt[:, k, :]

            if last:
                # xr first: the last block's router matmul runs early,
                # off the end-of-kernel critical path
                load_xr(b, nc.scalar)
                for i, c0 in enumerate(range(0, KC, pn)):
                    add(c0, pn, engs[i % 2])
            elif first:
                load_w1t(nc.sync, 0, 6)
                add(0, 3, nc.sync)
                add(3, 3, nc.scalar)
                load_w1t(nc.scalar, 6, 6)
                emit_small_consts(nc.scalar)
                add(6, 6, nc.scalar)
                add(12, 6, nc.sync)
                load_w1t(nc.sync, 12, 6)
                load_w1t(nc.scalar, 18, 6)
                add(18, 6, nc.scalar)
                load_xr(b, nc.scalar)
            else:
                for i, c0 in enumerate(range(0, KC, pn)):
                    add(c0, pn, engs[i % 2])
                load_xr(b, nc.scalar)

        def emit_router(b):
            mb = B_PLAN[b]
            gps = ps_gate.tile([E, 512], F32, tag="gate", name="gate")
            g = gps[:, 0:mb]
            xh, xl = st[b]["xh"], st[b]["xl"]
            # gate = whi@hi + wlo@hi + whi@lo
            nc.tensor.matmul(g, wr_sb[:], xh, start=True, stop=False)
            nc.tensor.matmul(g, wr2_sb[:], xh, start=False, stop=False)
            nc.tensor.matmul(g, wr_sb[:], xl, start=False, stop=True)
            st[b]["gps"] = gps

        def emit_burst0(b, nchunks=KC):
            # l1: bias row + first nchunks accumulating matmuls into PSUM
            mb = B_PLAN[b]
            cmap = st[b]["cmap"]
            ps1 = ps_big.tile([128, 512], F32, tag="ps1")
            st[b]["ps1"] = ps1
            for c in range(nchunks):
                nc.tensor.matmul(
                    ps1[:, 0:mb], w1t_sb[:, c, :], cmap[c], start=(c == 0), stop=False
                )
            st[b]["next_c"] = nchunks

        def emit_burst1(b):
            mb = B_PLAN[b]
            cmap = st[b]["cmap"]
            ps1 = st[b]["ps1"]
            for c in range(st[b]["next_c"], KC):
                nc.tensor.matmul(
                    ps1[:, 0:mb], w1t_sb[:, c, :], cmap[c],
                    start=False, stop=(c == KC - 1),
                )

        def emit_tail(b):
            mb, nsub = B_PLAN[b], B_PLAN[b] // 128
            noff = noffs[b]
            ps1 = st[b]["ps1"][:, 0:mb]
            # elementwise chains run on the Activation engine (otherwise
            # idle: all DMA issue is front-loaded), DVE only does the final
            # min()s (fp16 2x mode) and the argmax select
            # sq = min(1, (s*(l1c+b))^2); raw = min(1, max(0, l1c+b))
            AF = mybir.ActivationFunctionType
            sq = actp.tile([128, 512], F16, tag="sq", name="sq")[:, 0:mb]
            nc.scalar.activation(
                sq, ps1, AF.Square, bias=bias_sb[:, 0:1], scale=SQ_SCALE
            )
            raw = actp.tile([128, 512], F16, tag="raw", name="raw")[:, 0:mb]
            nc.scalar.activation(raw, ps1, AF.Relu, bias=bias_sb[:, 1:2])
            # pack rows 0:8 = gate + router_b; rows 32:40 get all_outputs
            # later; one PE transpose moves both to batch-major
            pack = smallp.tile([40, 512], F32, tag="pack", name="pack")[:, 0:mb]
            nc.scalar.activation(
                pack[0:8, :], st[b]["gps"][:, 0:mb], AF.Identity,
                bias=bias_sb[0:8, 5:6],
            )
            # lout = l1x_out + (l1_b15 + out_b), off the post-l2 chain
            lout = smallp.tile([E, 512], F32, tag="lout", name="lout")[:, 0:mb]
            nc.scalar.activation(
                lout, ps1[0:8, :], AF.Identity, bias=bias_sb[0:8, 4:5]
            )
            nc.vector.tensor_scalar_min(sq, sq, 1.0)
            nc.vector.tensor_scalar_min(raw, raw, 1.0)

            # l2: two expert groups, each sq+raw accumulated
            ps2a = ps_2a.tile([128, 512], F32, tag="ps2a", name="ps2a")[:, 0:mb]
            nc.tensor.matmul(ps2a, w2_sb[:, 0:128], sq, start=True, stop=False)
            nc.tensor.matmul(ps2a, w2_sb[:, 128:256], raw, start=False, stop=True)
            ps2b = ps_2b.tile([128, 512], F32, tag="ps2b", name="ps2b")[:, 0:mb]
            nc.tensor.matmul(ps2b, w2_sb[:, 256:384], sq, start=True, stop=False)
            nc.tensor.matmul(ps2b, w2_sb[:, 384:512], raw, start=False, stop=True)

            l2a = actp.tile([128, 512], F16, tag="l2a", name="l2a")[:, 0:mb]
            nc.scalar.activation(l2a, ps2a, AF.Relu, bias=bias_sb[:, 2:3])
            nc.vector.tensor_scalar_min(l2a, l2a, 1.0)
            l2b = actp.tile([128, 512], F16, tag="l2b", name="l2b")[:, 0:mb]
            nc.scalar.activation(l2b, ps2b, AF.Relu, bias=bias_sb[:, 3:4])
            nc.vector.tensor_scalar_min(l2b, l2b, 1.0)

            # l3: both groups accumulate into [8, mb]
            ps3 = ps_3.tile([E, 512], F32, tag="ps3", name="ps3")[:, 0:mb]
            nc.tensor.matmul(ps3, w3_sb[:, 0:8], l2a, start=True, stop=False)
            nc.tensor.matmul(ps3, w3_sb[:, 8:16], l2b, start=False, stop=True)

            # all_outputs.T = l3c + lout -> pack rows 32:40
            nc.vector.tensor_tensor(pack[32:40, :], ps3, lout, op=ALU.add)

            # single transpose per 128-chunk: out cols 0:8 gate.T, 32:40 all.T
            tb = ps_t.tile([128, 4, 40], F32, tag="tb", name="tb")
            for j in range(nsub):
                nc.tensor.transpose(
                    tb[:, j, :], pack[:, j * 128 : (j + 1) * 128], idn_sb[:]
                )
            gbt, abt = tb[:, :, 0:8], tb[:, :, 32:40]

            # argmax-select
            mx = smallp.tile([128, 4], F32, tag="mx", name="mx")[:, 0:nsub]
            nc.vector.reduce_max(mx, gbt[:, 0:nsub, :], axis=mybir.AxisListType.X)
            eq = smallp.tile([128, 4, E], F32, tag="eq", name="eq")
            for j in range(nsub):
                nc.vector.tensor_scalar(
                    eq[:, j, :], gbt[:, j, :], mx[:, j : j + 1], None, op0=ALU.is_ge
                )
            prod = smallp.tile([128, 4, E], F32, tag="prod", name="prod")[:, 0:nsub, :]
            nc.vector.tensor_tensor(prod, eq[:, 0:nsub, :], abt[:, 0:nsub, :], op=ALU.mult)
            yt = smallp.tile([128, 4], F32, tag="yt", name="yt")[:, 0:nsub]
            nc.vector.reduce_sum(yt, prod, axis=mybir.AxisListType.X)
            nc.sync.dma_start(y[:, noff : noff + nsub], yt)
            del st[b]

        # --- pipeline ---
        # all x pieces are SBUF-resident and issued up front, so the DMA
        # queues never stall on tile rotation; each block's l1 matmuls are
        # split around the previous block's tail so the PE has fill work
        # while chunks stream in.
        emit_load(0, first=True)
        for b in range(1, nb):
            emit_load(b, last=(b == nb - 1))
        for b in range(nb):
            if b == nb - 1:
                emit_router(b)  # xr arrived long ago; off the critical path
                emit_burst0(b, nchunks=12)
                emit_tail(b - 1)
                emit_burst1(b)
            else:
                emit_burst0(b, nchunks=12)
                if b > 0:
                    emit_tail(b - 1)
                emit_burst1(b)
                emit_router(b)
        emit_tail(nb - 1)

    nc.finalize()
    return nc


def prep_weights(router_w, router_b, l1_w, l1_b, l2_w, l2_b, out_w, out_b):
    """Host-side packing of the (tiny) weights into the kernel's layouts."""
    f4, f2 = np.float32, np.float16
    # W1 stacked: row f = e for o=15 (l1x_out), f = 8 + o*8 + e for o < 15
    w1_stacked = np.concatenate(
        [l1_w[:, L2, :], np.transpose(l1_w[:, :L2, :], (1, 0, 2)).reshape(120, L1)],
        axis=0,
    )  # [128, L1]
    w1t_kf = np.ascontiguousarray(w1_stacked.T).astype(f2)  # [L1, 128]
    # swizzle to [p, c, f] so the on-chip load is one fully contiguous DMA
    w1t = np.ascontiguousarray(
        np.transpose(w1t_kf.reshape(KC, 128, 128), (1, 0, 2))
    ).reshape(128, KC * 128)
    # l2 block weights: rows f_in = 8+o*8+e, packed [sqA | rawA | sqB | rawB]
    w2p = np.zeros((128, 512), f2)
    for e in range(E):
        base = 0 if e < 4 else 256
        c0 = (e % 4) * 32
        wt = l2_w[e].T  # [30, 32]; rows 0..14 sq features, 15..29 raw
        rows = 8 + np.arange(L2) * 8 + e  # f for o in 0..14
        w2p[rows, base + c0 : base + c0 + 32] = wt[0:L2]
        w2p[rows, base + 128 + c0 : base + 128 + c0 + 32] = wt[L2 : 2 * L2]
    # l3: [128, 16] = [W3A | W3B], each block [128, 8] with out partition = e.
    # Block A covers experts 0..3 (cols 4..7 zero), block B experts 4..7
    # (cols 0..3 zero) — out partition index is relative to the sliced lhsT.
    w3p = np.zeros((128, 16), f2)
    for e in range(E):
        col = e if e < 4 else 8 + e
        w3p[(e % 4) * 32 : (e % 4) * 32 + 32, col] = out_w[e, 0, :]
    # router weights, fp16 hi/lo split: gate = whi@hi + wlo@hi + whi@lo
    wT = np.ascontiguousarray(router_w.T).astype(f4)  # [64, 8]
    whi = wT.astype(f2)
    wlo = (wT - whi.astype(f4)).astype(f2)
    wrp = whi  # [64, 8]
    wr2 = wlo  # [64, 8]
    # bias columns
    b1col = np.concatenate([l1_b[:, L2], l1_b[:, :L2].T.reshape(120)])
    biasp = np.zeros((128, 6), f4)
    biasp[:, 0] = SQ_SCALE * b1col
    biasp[:, 1] = b1col
    biasp[:, 2] = l2_b[0:4].reshape(128)
    biasp[:, 3] = l2_b[4:8].reshape(128)
    biasp[0:8, 4] = l1_b[:, L2] + out_b[:, 0]
    biasp[0:8, 5] = router_b
    idn = np.eye(40, dtype=f4)
    return {
        "w1t": w1t, "w2p": w2p, "w3p": w3p,
        "wrp": wrp, "wr2": wr2, "biasp": biasp, "idn": idn,
    }


_cache = {}
_last_results = None


def kernel(x, router_w, router_b, l1_w, l1_b, l2_w, l2_b, out_w, out_b):
    global _last_results
    x = np.asarray(x, dtype=np.float32)
    weights = prep_weights(
        np.asarray(router_w, np.float32),
        np.asarray(router_b, np.float32),
        np.asarray(l1_w, np.float32),
        np.asarray(l1_b, np.float32),
        np.asarray(l2_w, np.float32),
        np.asarray(l2_b, np.float32),
        np.asarray(out_w, np.float32),
        np.asarray(out_b, np.float32),
    )

    xT_full = np.ascontiguousarray(x.T.astype(np.float16))  # [L1, B] fp16
    # router features, fp16 hi/lo split: [hi(64) ; lo(64)] on partitions
    rin = np.concatenate(
        [x[:, :RF], x[:, HALF : HALF + RF]], axis=1
    ).T.astype(np.float32)  # [64, B]
    rhi = rin.astype(np.float16)
    rlo = (rin - rhi.astype(np.float32)).astype(np.float16)
    xr_full = np.ascontiguousarray(np.concatenate([rhi, rlo], axis=0))  # [128, B]

    in_maps = []
    for c in range(N_CORES):
        sl = slice(c * B_SH, (c + 1) * B_SH)
        in_maps.append(
            {
                "xT": np.ascontiguousarray(xT_full[:, sl]),
                "xr": np.ascontiguousarray(xr_full[:, sl]),
                **weights,
            }
        )

    if "nc" not in _cache:
        _cache["nc"] = build_nc()
    nc = _cache["nc"]

    from concourse.bass_utils import run_bass_kernel_spmd

    trace = bool(int(os.environ.get("KERNEL_TRACE", "0")))
    try:
        res = run_bass_kernel_spmd(
            nc, in_maps, core_ids=list(range(N_CORES)), trace=trace
        )
    except Exception:
        if not trace:
            raise
        res = run_bass_kernel_spmd(
            nc, in_maps, core_ids=list(range(N_CORES)), trace=False
        )
    _last_results = res
    out = np.concatenate(
        [np.ascontiguousarray(r["y"].T).reshape(B_SH, 1) for r in res.results], axis=0
    )
    return out


# revision 22
# speedup vs baseline: 1.2342x; 1.2342x over previous
"""Trainium2 Bass kernel for nn_MoELayerStacks (moe_routing).

Full inputs in, full output out. Data-parallel over batch across 8 cores.

Math (per batch row b):
  gate = [x[:32], x[1536:1568]] @ router_w.T + router_b           # [8]
  idx  = argmax(gate)
  l1c  = x @ l1_w[e].T + l1_b[e]   for all e                      # [8, 16]
  l1x  = clip([square(l1c[:, :15])*255/256, l1c[:, :15]], 0, 1)   # [8, 30]
  l2x  = clip(l1x @ l2_w[e].T + l2_b[e], 0, 1)                    # [8, 32]
  out  = (l2x @ out_w[e].T + out_b[e] + l1c[:, 15])[idx]          # [1]

The problem is HBM-bound on reading x (201 MB fp32), so x and all matmul
weights are downcast to fp16 on the host: halves DMA traffic and runs the
PE at full rate (1 cyc/row). Verified rel-err ~3e-4 (budget 2e-2).

The router must match fp32 argmax bit-for-bit, so the 64 router features
are shipped as an fp16 hi/lo split (hi = fp16(x), lo = fp16(x - hi)) and
the gate is computed as whi@hi + wlo@hi + whi@lo in fp32 PSUM — max gate
error ~2e-6, zero argmax flips, same DMA bytes as one fp32 copy.

Layout: features on partitions ("transposed"), batch on the free dim.
Stacked l1 feature index: f = e for o=15 (the l1x_out features), and
f = 8 + o*8 + e for o in 0..14. l1's bias is folded into the PSUM
accumulation via a ones-row matmul. l2 outputs are split into two expert
groups (0-3, 4-7) of 128 features. The final argmax-gather runs in
batch-on-partitions layout after small PE transposes of the [8, mb]
gate / all-expert-output tiles.

Block plan per core: [512, 512, 512, 256, 256] batch rows. Each block's
l1 chunk matmuls are split around the previous block's tail so the PE has
fill work while DMA streams; the two trailing 256-blocks shrink the
serial compute tail after the last x byte lands.
"""

import os
from contextlib import ExitStack

import numpy as np

import concourse.bacc as bacc
import concourse.mybir as mybir
import concourse.tile as tile

N_CORES = 8
B, L1, L2, L3, E = 16384, 3072, 15, 32, 8
RF = 32  # router feats per perspective
HALF = L1 // 2
B_SH = B // N_CORES  # 2048 rows per core
KC = L1 // 128  # 24 contraction chunks
SQ_SCALE = float(np.sqrt(255.0 / 256.0))  # pre-square scale on l1 features
B_PLAN = [512, 512, 512, 256, 256]  # per-core batch blocks

F32 = mybir.dt.float32
F16 = mybir.dt.float16
ALU = mybir.AluOpType


def build_nc():
    nc = bacc.Bacc(dynamic_dma_scratch_size=2048)

    xT = nc.dram_tensor("xT", [L1, B_SH], F16, kind="ExternalInput")
    xr = nc.dram_tensor("xr", [128, B_SH], F16, kind="ExternalInput")
    w1t = nc.dram_tensor("w1t", [128, KC * 128], F16, kind="ExternalInput")
    wrp = nc.dram_tensor("wrp", [64, E], F16, kind="ExternalInput")
    wr2 = nc.dram_tensor("wr2", [64, E], F16, kind="ExternalInput")
    w2p = nc.dram_tensor("w2p", [128, 512], F16, kind="ExternalInput")
    w3p = nc.dram_tensor("w3p", [128, 16], F16, kind="ExternalInput")
    biasp = nc.dram_tensor("biasp", [128, 6], F32, kind="ExternalInput")
    idn = nc.dram_tensor("idn", [40, 40], F32, kind="ExternalInput")
    y = nc.dram_tensor("y", [128, B_SH // 128], F32, kind="ExternalOutput")

    moffs = np.cumsum([0] + B_PLAN).tolist()  # batch-row offset per block
    noffs = [m // 128 for m in moffs]  # y column offset per block
    nb = len(B_PLAN)

    with tile.TileContext(nc) as tc, ExitStack() as ctx:
        const = ctx.enter_context(tc.tile_pool(name="const", bufs=1))
        xp512 = ctx.enter_context(tc.tile_pool(name="xp512", bufs=12))
        xp256 = ctx.enter_context(tc.tile_pool(name="xp256", bufs=12))
        xrp = ctx.enter_context(tc.tile_pool(name="xrp", bufs=5))
        actp = ctx.enter_context(tc.tile_pool(name="act", bufs=2))
        smallp = ctx.enter_context(tc.tile_pool(name="small", bufs=2))
        ps_big = ctx.enter_context(tc.tile_pool(name="ps1", bufs=2, space="PSUM"))
        ps_gate = ctx.enter_context(tc.tile_pool(name="psg", bufs=2, space="PSUM"))
        ps_2a = ctx.enter_context(tc.tile_pool(name="ps2a", bufs=1, space="PSUM"))
        ps_2b = ctx.enter_context(tc.tile_pool(name="ps2b", bufs=1, space="PSUM"))
        ps_3 = ctx.enter_context(tc.tile_pool(name="ps3", bufs=1, space="PSUM"))
        ps_t = ctx.enter_context(tc.tile_pool(name="pst", bufs=1, space="PSUM"))

        # --- constants ---
        w1t_sb = const.tile([128, KC, 128], F16)
        w1t_v = w1t[:, :].rearrange("p (c f) -> p c f", f=128)

        def load_w1t(eng, c0, n):
            eng.dma_start(w1t_sb[:, c0 : c0 + n, :], w1t_v[:, c0 : c0 + n, :])

        w2_sb = const.tile([128, 512], F16)
        w3_sb = const.tile([128, 16], F16)
        wr_sb = const.tile([64, E], F16)
        wr2_sb = const.tile([64, E], F16)
        bias_sb = const.tile([128, 6], F32)
        idn_sb = const.tile([40, 40], F32)

        def emit_small_consts(eng):
            eng.dma_start(wr_sb[:], wrp[:, :])
            eng.dma_start(wr2_sb[:], wr2[:, :])
            eng.dma_start(w2_sb[:], w2p[:, :])
            eng.dma_start(w3_sb[:], w3p[:, :])
            eng.dma_start(bias_sb[:], biasp[:, :])
            eng.dma_start(idn_sb[:], idn[:, :])

        st = {}  # per-block live tiles for the skewed pipeline

        def load_xr(b, eng):
            # hi and lo in separate partition-0 tiles: the PE cannot take
            # matmul inputs at a nonzero partition offset (HW limitation)
            mb, m0 = B_PLAN[b], moffs[b]
            xh = xrp.tile([64, 512], F16, tag="xh", name="xh")[:, 0:mb]
            eng.dma_start(xh, xr[0:64, m0 : m0 + mb])
            xl = xrp.tile([64, 512], F16, tag="xl", name="xl")[:, 0:mb]
            eng.dma_start(xl, xr[64:128, m0 : m0 + mb])
            st[b]["xh"], st[b]["xl"] = xh, xl

        def load_piece(pool, tag, m0, mb, c0, n, nmax, eng):
            xt = pool.tile([128, nmax, mb], F16, tag=tag, name=tag)
            eng.dma_start(
                xt[:, 0:n, :],
                xT[c0 * 128 : (c0 + n) * 128, m0 : m0 + mb].rearrange(
                    "(c p) m -> p c m", p=128
                ),
            )
            return xt

        engs = [nc.sync, nc.sync]

        def emit_load(b, first=False, last=False):
            mb, m0 = B_PLAN[b], moffs[b]
            st[b] = {"cmap": [None] * KC}

            if mb == 512:
                pool, tag, pn = xp512, "xt", 6
            else:
                pool, tag, pn = xp256, "xt2", (3 if last else 6)

            def add(c0, n, eng):
                xt = load_piece(pool, tag, m0, mb, c0, n, 6, eng)
                for k in range(n):
                    st[b]["cmap"][c0 + k] = xt[:, k, :]

            if last:
                # xr first: the last block's router matmul runs early,
                # off the end-of-kernel critical path
                load_xr(b, nc.sync)
                for i, c0 in enumerate(range(0, KC, pn)):
                    add(c0, pn, engs[i % 2])
            elif first:
                load_w1t(nc.sync, 0, 6)
                add(0, 3, nc.sync)
                add(3, 3, nc.sync)
                load_w1t(nc.sync, 6, 6)
                emit_small_consts(nc.sync)
                add(6, 6, nc.sync)
                add(12, 6, nc.sync)
                load_w1t(nc.sync, 12, 6)
                load_w1t(nc.sync, 18, 6)
                add(18, 6, nc.sync)
                load_xr(b, nc.sync)
            else:
                for i, c0 in enumerate(range(0, KC, pn)):
                    add(c0, pn, engs[i % 2])
                load_xr(b, nc.sync)

        def emit_router(b):
            mb = B_PLAN[b]
            gps = ps_gate.tile([E, 512], F32, tag="gate", name="gate")
            g = gps[:, 0:mb]
            xh, xl = st[b]["xh"], st[b]["xl"]
            # gate = whi@hi + wlo@hi + whi@lo
            nc.tensor.matmul(g, wr_sb[:], xh, start=True, stop=False)
            nc.tensor.matmul(g, wr2_sb[:], xh, start=False, stop=False)
            nc.tensor.matmul(g, wr_sb[:], xl, start=False, stop=True)
            st[b]["gps"] = gps

        def emit_burst0(b, nchunks=KC):
            # l1: bias row + first nchunks accumulating matmuls into PSUM
            mb = B_PLAN[b]
            cmap = st[b]["cmap"]
            ps1 = ps_big.tile([128, 512], F32, tag="ps1")
            st[b]["ps1"] = ps1
            for c in range(nchunks):
                nc.tensor.matmul(
                    ps1[:, 0:mb], w1t_sb[:, c, :], cmap[c], start=(c == 0), stop=False
                )
            st[b]["next_c"] = nchunks

        def emit_burst1(b):
            mb = B_PLAN[b]
            cmap = st[b]["cmap"]
            ps1 = st[b]["ps1"]
            for c in range(st[b]["next_c"], KC):
                nc.tensor.matmul(
                    ps1[:, 0:mb], w1t_sb[:, c, :], cmap[c],
                    start=False, stop=(c == KC - 1),
                )

        def emit_tail(b):
            mb, nsub = B_PLAN[b], B_PLAN[b] // 128
            noff = noffs[b]
            ps1 = st[b]["ps1"][:, 0:mb]
            # elementwise chains run on the Activation engine (otherwise
            # idle: all DMA issue is front-loaded), DVE only does the final
            # min()s (fp16 2x mode) and the argmax select
            # sq = min(1, (s*(l1c+b))^2); raw = min(1, max(0, l1c+b))
            AF = mybir.ActivationFunctionType
            sq = actp.tile([128, 512], F16, tag="sq", name="sq")[:, 0:mb]
            nc.scalar.activation(
                sq, ps1, AF.Square, bias=bias_sb[:, 0:1], scale=SQ_SCALE
            )
            raw = actp.tile([128, 512], F16, tag="raw", name="raw")[:, 0:mb]
            nc.scalar.activation(raw, ps1, AF.Relu, bias=bias_sb[:, 1:2])
            # pack rows 0:8 = gate + router_b; rows 32:40 get all_outputs
            # later; one PE transpose moves both to batch-major
            pack = smallp.tile([40, 512], F32, tag="pack", name="pack")[:, 0:mb]
            nc.scalar.activation(
                pack[0:8, :], st[b]["gps"][:, 0:mb], AF.Identity,
                bias=bias_sb[0:8, 5:6],
            )
            # lout = l1x_out + (l1_b15 + out_b), off the post-l2 chain
            lout = smallp.tile([E, 512], F32, tag="lout", name="lout")[:, 0:mb]
            nc.scalar.activation(
                lout, ps1[0:8, :], AF.Identity, bias=bias_sb[0:8, 4:5]
            )
            nc.vector.tensor_scalar_min(sq, sq, 1.0)
            nc.vector.tensor_scalar_min(raw, raw, 1.0)

            # l2: two expert groups, each sq+raw accumulated
            ps2a = ps_2a.tile([128, 512], F32, tag="ps2a", name="ps2a")[:, 0:mb]
            nc.tensor.matmul(ps2a, w2_sb[:, 0:128], sq, start=True, stop=False)
            nc.tensor.matmul(ps2a, w2_sb[:, 128:256], raw, start=False, stop=True)
            ps2b = ps_2b.tile([128, 512], F32, tag="ps2b", name="ps2b")[:, 0:mb]
            nc.tensor.matmul(ps2b, w2_sb[:, 256:384], sq, start=True, stop=False)
            nc.tensor.matmul(ps2b, w2_sb[:, 384:512], raw, start=False, stop=True)

            l2a = actp.tile([128, 512], F16, tag="l2a", name="l2a")[:, 0:mb]
            nc.scalar.activation(l2a, ps2a, AF.Relu, bias=bias_sb[:, 2:3])
            nc.vector.tensor_scalar_min(l2a, l2a, 1.0)
            l2b = actp.tile([128, 512], F16, tag="l2b", name="l2b")[:, 0:mb]
            nc.scalar.activation(l2b, ps2b, AF.Relu, bias=bias_sb[:, 3:4])
            nc.vector.tensor_scalar_min(l2b, l2b, 1.0)

            # l3: both groups accumulate into [8, mb]
            ps3 = ps_3.tile([E, 512], F32, tag="ps3", name="ps3")[:, 0:mb]
            nc.tensor.matmul(ps3, w3_sb[:, 0:8], l2a, start=True, stop=False)
            nc.tensor.matmul(ps3, w3_sb[:, 8:16], l2b, start=False, stop=True)

            # all_outputs.T = l3c + lout -> pack rows 32:40
            nc.vector.tensor_tensor(pack[32:40, :], ps3, lout, op=ALU.add)

            # single transpose per 128-chunk: out cols 0:8 gate.T, 32:40 all.T
            tb = ps_t.tile([128, 4, 40], F32, tag="tb", name="tb")
            for j in range(nsub):
                nc.tensor.transpose(
                    tb[:, j, :], pack[:, j * 128 : (j + 1) * 128], idn_sb[:]
                )
            gbt, abt = tb[:, :, 0:8], tb[:, :, 32:40]

            # argmax-select
            mx = smallp.tile([128, 4], F32, tag="mx", name="mx")[:, 0:nsub]
            nc.vector.reduce_max(mx, gbt[:, 0:nsub, :], axis=mybir.AxisListType.X)
            eq = smallp.tile([128, 4, E], F32, tag="eq", name="eq")
            for j in range(nsub):
                nc.vector.tensor_scalar(
                    eq[:, j, :], gbt[:, j, :], mx[:, j : j + 1], None, op0=ALU.is_ge
                )
            prod = smallp.tile([128, 4, E], F32, tag="prod", name="prod")[:, 0:nsub, :]
            nc.vector.tensor_tensor(prod, eq[:, 0:nsub, :], abt[:, 0:nsub, :], op=ALU.mult)
            yt = smallp.tile([128, 4], F32, tag="yt", name="yt")[:, 0:nsub]
            nc.vector.reduce_sum(yt, prod, axis=mybir.AxisListType.X)
            nc.sync.dma_start(y[:, noff : noff + nsub], yt)
            del st[b]

        # --- pipeline ---
        # all x pieces are SBUF-resident and issued up front, so the DMA
        # queues never stall on tile rotation; each block's l1 matmuls are
        # split around the previous block's tail so the PE has fill work
        # while chunks stream in.
        emit_load(0, first=True)
        for b in range(1, nb):
            emit_load(b, last=(b == nb - 1))
        for b in range(nb):
            if b == nb - 1:
                emit_router(b)  # xr arrived long ago; off the critical path
                emit_burst0(b, nchunks=12)
                emit_tail(b - 1)
                emit_burst1(b)
            else:
                emit_burst0(b, nchunks=12)
                if b > 0:
                    emit_tail(b - 1)
                emit_burst1(b)
                emit_router(b)
        emit_tail(nb - 1)

    nc.finalize()
    return nc


def prep_weights(router_w, router_b, l1_w, l1_b, l2_w, l2_b, out_w, out_b):
    """Host-side packing of the (tiny) weights into the kernel's layouts."""
    f4, f2 = np.float32, np.float16
    # W1 stacked: row f = e for o=15 (l1x_out), f = 8 + o*8 + e for o < 15
    w1_stacked = np.concatenate(
        [l1_w[:, L2, :], np.transpose(l1_w[:, :L2, :], (1, 0, 2)).reshape(120, L1)],
        axis=0,
    )  # [128, L1]
    w1t_kf = np.ascontiguousarray(w1_stacked.T).astype(f2)  # [L1, 128]
    # swizzle to [p, c, f] so the on-chip load is one fully contiguous DMA
    w1t = np.ascontiguousarray(
        np.transpose(w1t_kf.reshape(KC, 128, 128), (1, 0, 2))
    ).reshape(128, KC * 128)
    # l2 block weights: rows f_in = 8+o*8+e, packed [sqA | rawA | sqB | rawB]
    w2p = np.zeros((128, 512), f2)
    for e in range(E):
        base = 0 if e < 4 else 256
        c0 = (e % 4) * 32
        wt = l2_w[e].T  # [30, 32]; rows 0..14 sq features, 15..29 raw
        rows = 8 + np.arange(L2) * 8 + e  # f for o in 0..14
        w2p[rows, base + c0 : base + c0 + 32] = wt[0:L2]
        w2p[rows, base + 128 + c0 : base + 128 + c0 + 32] = wt[L2 : 2 * L2]
    # l3: [128, 16] = [W3A | W3B], each block [128, 8] with out partition = e.
    # Block A covers experts 0..3 (cols 4..7 zero), block B experts 4..7
    # (cols 0..3 zero) — out partition index is relative to the sliced lhsT.
    w3p = np.zeros((128, 16), f2)
    for e in range(E):
        col = e if e < 4 else 8 + e
        w3p[(e % 4) * 32 : (e % 4) * 32 + 32, col] = out_w[e, 0, :]
    # router weights, fp16 hi/lo split: gate = whi@hi + wlo@hi + whi@lo
    wT = np.ascontiguousarray(router_w.T).astype(f4)  # [64, 8]
    whi = wT.astype(f2)
    wlo = (wT - whi.astype(f4)).astype(f2)
    wrp = whi  # [64, 8]
    wr2 = wlo  # [64, 8]
    # bias columns
    b1col = np.concatenate([l1_b[:, L2], l1_b[:, :L2].T.reshape(120)])
    biasp = np.zeros((128, 6), f4)
    biasp[:, 0] = SQ_SCALE * b1col
    biasp[:, 1] = b1col
    biasp[:, 2] = l2_b[0:4].reshape(128)
    biasp[:, 3] = l2_b[4:8].reshape(128)
    biasp[0:8, 4] = l1_b[:, L2] + out_b[:, 0]
    biasp[0:8, 5] = router_b
    idn = np.eye(40, dtype=f4)
    return {
        "w1t": w1t, "w2p": w2p, "w3p": w3p,
        "wrp": wrp, "wr2": wr2, "biasp": biasp, "idn": idn,
    }


_cache = {}
_last_results = None


def kernel(x, router_w, router_b, l1_w, l1_b, l2_w, l2_b, out_w, out_b):
    global _last_results
    x = np.asarray(x, dtype=np.float32)
    weights = prep_weights(
        np.asarray(router_w, np.float32),
        np.asarray(router_b, np.float32),
        np.asarray(l1_w, np.float32),
        np.asarray(l1_b, np.float32),
        np.asarray(l2_w, np.float32),
        np.asarray(l2_b, np.float32),
        np.asarray(out_w, np.float32),
        np.asarray(out_b, np.float32),
    )

    xT_full = np.ascontiguousarray(x.T.astype(np.float16))  # [L1, B] fp16
    # router features, fp16 hi/lo split: [hi(64) ; lo(64)] on partitions
    rin = np.concatenate(
        [x[:, :RF], x[:, HALF : HALF + RF]], axis=1
    ).T.astype(np.float32)  # [64, B]
    rhi = rin.astype(np.float16)
    rlo = (rin - rhi.astype(np.float32)).astype(np.float16)
    xr_full = np.ascontiguousarray(np.concatenate([rhi, rlo], axis=0))  # [128, B]

    in_maps = []
    for c in range(N_CORES):
        sl = slice(c * B_SH, (c + 1) * B_SH)
        in_maps.append(
            {
                "xT": np.ascontiguousarray(xT_full[:, sl]),
                "xr": np.ascontiguousarray(xr_full[:, sl]),
                **weights,
            }
        )

    if "nc" not in _cache:
        _cache["nc"] = build_nc()
    nc = _cache["nc"]

    from concourse.bass_utils import run_bass_kernel_spmd

    trace = bool(int(os.environ.get("KERNEL_TRACE", "0")))
    try:
        res = run_bass_kernel_spmd(
            nc, in_maps, core_ids=list(range(N_CORES)), trace=trace
        )
    except Exception:
        if not trace:
            raise
        res = run_bass_kernel_spmd(
            nc, in_maps, core_ids=list(range(N_CORES)), trace=False
        )
    _last_results = res
    out = np.concatenate(
        [np.ascontiguousarray(r["y"].T).reshape(B_SH, 1) for r in res.results], axis=0
    )
    return out
